# revision 40
# baseline (speedup 1.0000x reference)
"""Conformer block kernel for 8 Trainium2 NeuronCores.

Sharding: pure data-parallel over batch (B=8 -> 1 batch element per core,
zero collectives). All weights are replicated; BatchNorm affines and scalar
multipliers are folded into the adjacent pointwise-conv weights on the host.
All matmuls run as float32r (full PE rate, ~1e-4 rounding).
"""
import sys

sys.path.insert(0, '/opt/trn_rl_repo')

import numpy as np
import ml_dtypes
import concourse.bass as bass
import concourse.tile as tile
from concourse import bacc, mybir
from concourse.bass_utils import run_bass_kernel_spmd

F32 = mybir.dt.float32
F32R = mybir.dt.float32r
F8 = mybir.dt.float8e4
NPF8 = ml_dtypes.float8_e4m3
PM_DR = mybir.MatmulPerfMode.DoubleRow
AF = mybir.ActivationFunctionType
ALU = mybir.AluOpType

# fp8 scaling: u (GLU output) is scaled by U_SCALE (folded into the pw1 'a'
# half), dconv weights by W_SCALE; the Silu reading the conv PSUM divides
# both back out.
U_SCALE = 16.0
W_SCALE = 64.0
# generic weight-quantization scale for the other fp8 matmuls (divided back
# out via the activation `scale` when reading PSUM)
WQ = 64.0
DO_PW2_DR = False


def _pack_pairs(wp, wscale):
    """[Cin, Cout] f32 -> fp8 DoubleRow layout [Cin//256, 128, 2*Cout].

    Pairs two 128-row ci tiles per matmul: tile[cp][ci, two, co].
    """
    cin, cout = wp.shape
    t = wp.reshape(cin // 256, 2, 128, cout).transpose(0, 2, 1, 3)
    return np.ascontiguousarray((t * wscale).astype(NPF8)).reshape(
        cin // 256, 128, 2 * cout)

B, DIM, S = 8, 512, 1024
H, DH = 8, 64
FF_INNER = 1024
CONV_INNER = 1024
K = 31
PAD = (K - 1) // 2
N_CORES = 8

CT = DIM // 128          # 4  channel tiles of the 512-dim stream
UT = CONV_INNER // 128   # 8  tiles of 1024-wide inner dims
SC = S // 512            # 2  free-dim chunks of 512


def _host_prep(i):
    """Fold affines/scalars into weights; pre-transpose for lhsT layout."""
    f = np.float32
    w = {}
    w['x'] = None  # filled per core

    def fold(wmat, g, b, bout):
        # y = wmat @ (g*x + b) + bout  ->  W' = wmat * g[None, :],
        # b' = wmat @ b + bout ; return transposed W' [in, out]
        wp = (wmat * g[None, :]).astype(f)
        bp = (wmat @ b + bout).astype(f)
        return np.ascontiguousarray(wp.T), bp

    w1, w['b_ff1_1'] = fold(i['ff1_w1'], i['ff1_g'], i['ff1_b'], i['ff1_b1'])
    w['w_ff1_1'] = _pack_pairs(w1, WQ).reshape(2, 128, 2, FF_INNER)
    w['w_ff1_2'] = _pack_pairs(
        (0.5 * i['ff1_w2']).T.astype(f), WQ).reshape(4, 128, 2, DIM)
    w['b_ff1_2'] = (0.5 * i['ff1_b2']).astype(f)

    wq, w['b_q'] = fold(i['wq'], i['attn_g'], i['attn_b'], i['bq'])
    wk, w['b_k'] = fold(i['wk'], i['attn_g'], i['attn_b'], i['bk'])
    wv, w['b_v'] = fold(i['wv'], i['attn_g'], i['attn_b'], i['bv'])
    w['w_q'] = _pack_pairs(wq, WQ).reshape(2, 128, 2, DIM)
    w['w_k'] = _pack_pairs(wk, WQ).reshape(2, 128, 2, DIM)
    w['w_v'] = _pack_pairs(wv, WQ).reshape(2, 128, 2, DIM)
    w['w_o'] = np.ascontiguousarray(i['wo'].T.astype(f))
    w['b_o'] = i['bo'].astype(f)

    w['w_pw1'], w['b_pw1'] = fold(i['pw1_w'], i['conv_g'], i['conv_b'], i['pw1_b'])
    # scale the GLU 'a' half so the fp8 cast of u = a*sigmoid(g) lands
    # mid-range (divided back out after the dconv)
    w['w_pw1'][:, :CONV_INNER] *= U_SCALE
    w['b_pw1'][:CONV_INNER] *= U_SCALE
    # dconv: fold cbn_g into weights; bias = cbn_g*dconv_b + cbn_b
    dw = (i['dconv_w'][:, :, 0, :] * i['cbn_g'][:, None, None]).astype(f)  # [o,c,k]
    # fp8 DoubleRow layout: pair two 128-wide ci tiles per matmul.
    # [ot, cp, ci(128), k, two, co(128)]; lhsT slice per k = [128, 2, 128]
    dwt = dw.reshape(UT, 128, UT // 2, 2, 128, K)  # [ot, co, cp, two, ci, k]
    dwt = np.ascontiguousarray(dwt.transpose(0, 2, 4, 5, 3, 1))  # [ot,cp,ci,k,two,co]
    w['w_dc'] = (dwt * W_SCALE).astype(NPF8).reshape(UT, UT // 2, 128, K * 256)
    w['b_dc'] = (i['cbn_g'] * i['dconv_b'] + i['cbn_b']).astype(f)
    w['w_pw2'] = np.ascontiguousarray(i['pw2_w'].T.astype(f))
    w['b_pw2'] = i['pw2_b'].astype(f)

    w2, w['b_ff2_1'] = fold(i['ff2_w1'], i['ff2_g'], i['ff2_b'], i['ff2_b1'])
    w['w_ff2_1'] = _pack_pairs(w2, WQ).reshape(2, 128, 2, FF_INNER)
    w['w_ff2_2'] = _pack_pairs(
        (0.5 * i['ff2_w2']).T.astype(f), WQ).reshape(4, 128, 2, DIM)
    w['b_ff2_2'] = (0.5 * i['ff2_b2']).astype(f)

    # final affine folded into ff2's output stage:
    # out = x3*g + (psum*(g/WQ) + (g*b2 + b_fin))
    w['fin_g'] = i['fin_g'].astype(f)
    w['fin_sc'] = (i['fin_g'] / WQ).astype(f)
    w['fin_bb'] = (i['fin_g'] * w['b_ff2_2'] + i['fin_b']).astype(f)
    w['zpad'] = np.zeros(PAD, NPF8)
    del w['x']
    return w


def _bias_tile(nc, sb, dram_vec, n):
    """Load a [n*128] DRAM vector as a [128, n] SBUF tile (col t = tile t)."""
    t = sb.tile([128, n], F32, tag=f'bias_{dram_vec.name}', name=f'b_{dram_vec.name}')
    nc.sync.dma_start(t[:], dram_vec.ap().rearrange('(t p) -> p t', p=128))
    return t


def _bcast_tile(nc, sb, dram_vec, n, tag):
    """Broadcast a [n] DRAM vector across 128 partitions -> [128, n] f32r."""
    t = sb.tile([128, n], F32R, tag=tag, name=tag)
    v = dram_vec.ap()
    nc.sync.dma_start(
        t[:], bass.AP(tensor=v.tensor, offset=0, ap=[[0, 128], [1, n]]))
    return t


def build_program():
    nc = bacc.Bacc('TRN2', target_bir_lowering=False, debug=False)
    dt_in = {}

    def din(name, shape, dt=F32R):
        dt_in[name] = nc.dram_tensor(name, shape, dt, kind='ExternalInput')
        return dt_in[name]

    x_d = din('x', [DIM, S])
    w_ff1_1 = din('w_ff1_1', [2, 128, 2, FF_INNER], F8); b_ff1_1 = din('b_ff1_1', [FF_INNER], F32)
    w_ff1_2 = din('w_ff1_2', [4, 128, 2, DIM], F8); b_ff1_2 = din('b_ff1_2', [DIM], F32)
    w_q = din('w_q', [2, 128, 2, DIM], F8); b_q = din('b_q', [DIM], F32)
    w_k = din('w_k', [2, 128, 2, DIM], F8); b_k = din('b_k', [DIM], F32)
    w_v = din('w_v', [2, 128, 2, DIM], F8); b_v = din('b_v', [DIM])
    w_o = din('w_o', [DIM, DIM]); b_o = din('b_o', [DIM], F32)
    w_pw1 = din('w_pw1', [DIM, 2 * CONV_INNER]); b_pw1 = din('b_pw1', [2 * CONV_INNER], F32)
    w_dc = din('w_dc', [UT, UT // 2, 128, K * 256], F8)
    b_dc = din('b_dc', [CONV_INNER], F32)
    w_pw2 = din('w_pw2', [CONV_INNER, DIM]); b_pw2 = din('b_pw2', [DIM], F32)
    w_ff2_1 = din('w_ff2_1', [2, 128, 2, FF_INNER], F8); b_ff2_1 = din('b_ff2_1', [FF_INNER], F32)
    w_ff2_2 = din('w_ff2_2', [4, 128, 2, DIM], F8); b_ff2_2 = din('b_ff2_2', [DIM], F32)
    fin_g = din('fin_g', [DIM], F32)
    fin_sc = din('fin_sc', [DIM], F32); fin_bb = din('fin_bb', [DIM], F32)
    zpad = din('zpad', [PAD], F8)
    out_d = nc.dram_tensor('out', [DIM, S], F32, kind='ExternalOutput')

    with tile.TileContext(nc, pool_alloc_mode='queue') as tc:
        _emit(nc, tc, dt_in, out_d)
    nc.compile()
    return nc


def _emit(nc, tc, din, out_d):
    from contextlib import ExitStack
    rec_scr = nc.dram_tensor('rec_scratch', [2, 512], F32, kind='Internal')
    ctx = ExitStack()
    with ctx:
        # ---- persistent pools -------------------------------------------
        resid = ctx.enter_context(tc.tile_pool(name='resid', bufs=2))
        hid = ctx.enter_context(tc.tile_pool(name='hid', bufs=1))
        btp = ctx.enter_context(tc.tile_pool(name='biases', bufs=1))

        def new_resid(i):
            return resid.tile([128, S], F32R, tag=f'r{i}', name=f'r{i}')

        def hid_tile(i):
            return hid.tile([128, S], F32R, tag=f'h{i}', name=f'h{i}')

        # load x
        x_sb = []
        for i in range(CT):
            t = new_resid(i)
            nc.sync.dma_start(t[:], din['x'].ap()[i * 128:(i + 1) * 128, :])
            x_sb.append(t)

        # ---- generic FF macro (fp8 DoubleRow matmuls) -------------------
        def ff_block(x_in, w1d, b1d, w2d, b2d, nm, final=None):
            with tc.tile_pool(name=f'w{nm}', bufs=1) as wp, \
                 tc.tile_pool(name=f'x8{nm}', bufs=1) as xp, \
                 tc.tile_pool(name=f'tmp{nm}', bufs=3) as tpp, \
                 tc.tile_pool(name=f'ps{nm}', bufs=4, space='PSUM') as ps:
                w1_sb = []
                for i in range(2):
                    t = wp.tile([128, 2, FF_INNER], F8, tag=f'w1_{i}')
                    nc.sync.dma_start(t[:], w1d.ap()[i])
                    w1_sb.append(t)
                w2_sb = []
                for i in range(4):
                    t = wp.tile([128, 2, DIM], F8, tag=f'w2_{i}')
                    nc.sync.dma_start(t[:], w2d.ap()[i])
                    w2_sb.append(t)
                b1_t = _bias_tile(nc, btp, b1d, UT)
                b2_t = _bias_tile(nc, btp, b2d, CT)

                # cast block input to fp8 channel-tile pairs
                x8 = [xp.tile([128, 2, S], F8, tag=f'x8_{i}', name=f'x8_{i}') for i in range(2)]
                for ct in range(CT):
                    nc.scalar.activation(
                        x8[ct // 2][:, ct % 2, :], x_in[ct][:], AF.Copy)

                h8 = [xp.tile([128, 2, S], F8, tag=f'h8_{i}', name=f'h8_{i}') for i in range(4)]
                for ot in range(UT):
                    for sc in range(SC):
                        p = ps.tile([128, 512], F32, tag='pp')
                        for cp in range(2):
                            nc.tensor.matmul(
                                p[:], w1_sb[cp][:, :, ot * 128:(ot + 1) * 128],
                                x8[cp][:, :, sc * 512:(sc + 1) * 512],
                                start=(cp == 0), stop=(cp == 1),
                                perf_mode=PM_DR)
                        nc.scalar.activation(
                            h8[ot // 2][:, ot % 2, sc * 512:(sc + 1) * 512],
                            p[:], AF.Silu, bias=b1_t[:, ot:ot + 1],
                            scale=1.0 / WQ)
                x_out = []
                for ot in range(CT):
                    t = new_resid(ot) if final is None else None
                    for sc in range(SC):
                        ssl_ = slice(sc * 512, (sc + 1) * 512)
                        p = ps.tile([128, 512], F32, tag='pp')
                        for hp in range(4):
                            nc.tensor.matmul(
                                p[:], w2_sb[hp][:, :, ot * 128:(ot + 1) * 128],
                                h8[hp][:, :, sc * 512:(sc + 1) * 512],
                                start=(hp == 0), stop=(hp == 3),
                                perf_mode=PM_DR)
                        if final is None:
                            tmp = tpp.tile([128, 512], F32, tag='tmp')
                            nc.scalar.activation(
                                tmp[:], p[:], AF.Identity,
                                bias=b2_t[:, ot:ot + 1], scale=1.0 / WQ)
                            nc.vector.tensor_tensor(
                                t[:, ssl_], tmp[:], x_in[ot][:, ssl_],
                                op=ALU.add)
                        else:
                            # fused: out = x_in*g + (psum*(g/WQ) + (g*b2+b))
                            fg_t, fsc_t, fbb_t, out_dram = final
                            tmp = tpp.tile([128, 512], F32, tag='tmp')
                            nc.scalar.activation(
                                tmp[:], p[:], AF.Identity,
                                bias=fbb_t[:, ot:ot + 1],
                                scale=fsc_t[:, ot:ot + 1])
                            o_t = tpp.tile([128, 512], F32, tag='fin')
                            nc.vector.scalar_tensor_tensor(
                                o_t[:], x_in[ot][:, ssl_],
                                fg_t[:, ot:ot + 1], tmp[:],
                                op0=ALU.mult, op1=ALU.add)
                            nc.sync.dma_start(
                                out_dram.ap()[ot * 128:(ot + 1) * 128, ssl_],
                                o_t[:])
                    if t is not None:
                        x_out.append(t)
                return x_out

        # ================= FF1 =================
        x1_sb = ff_block(x_sb, din['w_ff1_1'], din['b_ff1_1'],
                         din['w_ff1_2'], din['b_ff1_2'], 'ff1')

        # attention weights (fp8, small) overlap with FF1 compute
        wattn = ctx.enter_context(tc.tile_pool(name='wattn', bufs=1))
        wq_sb, wk_sb, wv_sb = [], [], []
        for nm, lst in (('w_q', wq_sb), ('w_k', wk_sb), ('w_v', wv_sb)):
            for i in range(2):
                t = wattn.tile([128, 2, DIM], F8, tag=f'{nm}_{i}', name=f'{nm}_{i}')
                nc.sync.dma_start(t[:], din[nm].ap()[i])
                lst.append(t)
        wo_sb = []
        for i in range(CT):
            t = wattn.tile([128, DIM], F32R, tag=f'w_o_{i}', name=f'w_o_{i}')
            nc.sync.dma_start(t[:], din['w_o'].ap()[i * 128:(i + 1) * 128, :])
            wo_sb.append(t)
        bv_bc = _bcast_tile(nc, wattn, din['b_v'], DIM, 'bv_bc')
        bq_t = _bias_tile(nc, btp, din['b_q'], CT)
        bk_t = _bias_tile(nc, btp, din['b_k'], CT)
        bo_t = _bias_tile(nc, btp, din['b_o'], CT)

        # ================= Attention =================
        with tc.tile_pool(name='attn_sb', bufs=1) as asb, \
             tc.tile_pool(name='attn_e', bufs=6) as epool, \
             tc.tile_pool(name='attn_misc', bufs=3) as misc:
            # cast x1 to fp8 channel-tile pairs for the DR projections
            x18 = [asb.tile([128, 2, S], F8, tag=f'x18_{i}', name=f'x18_{i}') for i in range(2)]
            for ct in range(CT):
                nc.scalar.activation(
                    x18[ct // 2][:, ct % 2, :], x1_sb[ct][:], AF.Copy)
            # Q, K projections (fp8 DR), f32r outputs in hid slots
            q_sb = [hid_tile(i) for i in range(CT)]
            k_sb = [hid_tile(CT + i) for i in range(CT)]
            with tc.tile_pool(name='ps_proj', bufs=3, space='PSUM') as ppp:
                for dst, w_sb, b_t in ((q_sb, wq_sb, bq_t), (k_sb, wk_sb, bk_t)):
                    for ot in range(CT):
                        for sc in range(SC):
                            p = ppp.tile([128, 512], F32, tag='pp')
                            for cp in range(2):
                                nc.tensor.matmul(
                                    p[:], w_sb[cp][:, :, ot * 128:(ot + 1) * 128],
                                    x18[cp][:, :, sc * 512:(sc + 1) * 512],
                                    start=(cp == 0), stop=(cp == 1),
                                    perf_mode=PM_DR)
                            nc.scalar.activation(
                                dst[ot][:, sc * 512:(sc + 1) * 512], p[:],
                                AF.Identity, bias=b_t[:, ot:ot + 1],
                                scale=1.0 / WQ)
                # V transposed fp8 in t-tile pairs, interleaved per head as
                # [v_h (64 cols) | ones (64 cols)] so one DR matmul chain
                # yields both A@V (rows 0:64) and sum(exp) (rows 64:128).
                vt8 = [asb.tile([128, 2, 2 * DIM], F8, tag=f'vt{i}', name=f'vt{i}') for i in range(4)]
                for tp in range(4):
                    for sl in range(2):
                        ones_view = vt8[tp][:, sl, :].rearrange(
                            'p (h c) -> p h c', c=128)[:, :, 64:128]
                        nc.vector.memset(ones_view, 1.0)
                for tt in range(UT):
                    p = ppp.tile([128, 512], F32, tag='pp')
                    for cp in range(2):
                        nc.tensor.matmul(
                            p[:], x18[cp][:, :, tt * 128:(tt + 1) * 128],
                            wv_sb[cp][:], start=(cp == 0), stop=(cp == 1),
                            perf_mode=PM_DR)
                    v_view = vt8[tt // 2][:, tt % 2, :].rearrange(
                        'p (h c) -> p h c', c=128)[:, :, 0:64]
                    nc.vector.scalar_tensor_tensor(
                        v_view, p[:].rearrange('p (h c) -> p h c', c=64),
                        1.0 / WQ,
                        bv_bc[:].rearrange('p (h c) -> p h c', c=64),
                        op0=ALU.mult, op1=ALU.add)

            # per-head attention: heads are processed in pairs (h0=even at
            # partitions 0:64, h1=odd at 64:128) so the 64-deep contraction
            # score matmuls land in distinct PE row groups and run
            # concurrently. Each head's AV+sumexp is ONE DR matmul chain
            # via the interleaved [v|ones] lhsT.
            o_sb = [asb.tile([128, S], F32R, tag=f'o{i}', name=f'o{i}') for i in range(CT)]
            x2_sb = [new_resid(i) for i in range(CT)]
            with tc.tile_pool(name='ps_sc', bufs=2, space='PSUM') as psc, \
                 tc.tile_pool(name='ps_acc', bufs=2, space='PSUM') as pac:
                for sc in range(SC):
                    ssl = slice(sc * 512, (sc + 1) * 512)
                    for hp in range(CT):
                        slab = hp
                        q0, k0 = q_sb[slab][0:64, :], k_sb[slab][0:64, :]
                        q1, k1 = q_sb[slab][64:128, :], k_sb[slab][64:128, :]
                        p_o = [pac.tile([128, 512], F32, tag=f'po{j}',
                                        name=f'po{j}') for j in range(2)]
                        for tp in range(4):
                            e8 = [epool.tile([128, 2, 512], F8, tag=f'e{j}', name=f'e{j}')
                                  for j in range(2)]
                            for i in range(2):
                                tt = 2 * tp + i
                                tsl = slice(tt * 128, (tt + 1) * 128)
                                p0 = psc.tile([128, 512], F32, tag='sc0')
                                p1 = psc.tile([128, 512], F32, tag='sc1')
                                nc.tensor.matmul(p0[:], k0[:, tsl], q0[:, ssl],
                                                 start=True, stop=True)
                                nc.tensor.matmul(p1[:], k1[:, tsl], q1[:, ssl],
                                                 start=True, stop=True)
                                nc.scalar.activation(e8[0][:, i, :], p0[:],
                                                     AF.Exp,
                                                     scale=float(DH) ** -0.5)
                                nc.scalar.activation(e8[1][:, i, :], p1[:],
                                                     AF.Exp,
                                                     scale=float(DH) ** -0.5)
                            for j, h in ((0, 2 * hp), (1, 2 * hp + 1)):
                                nc.tensor.matmul(
                                    p_o[j][:],
                                    vt8[tp][:, :, h * 128:(h + 1) * 128],
                                    e8[j][:], start=(tp == 0), stop=(tp == 3),
                                    perf_mode=PM_DR)
                        for j in range(2):
                            po = 64 * j
                            # sumexp rows are 64 replicas: reciprocal ONE row,
                            # bounce it through DRAM to partition-broadcast
                            # (both DMAs on the gpsimd queue -> FIFO-safe)
                            s_rec = misc.tile([128, 512], F32, tag=f'st{j}',
                                              name=f'st{j}')
                            nc.vector.reciprocal(s_rec[64:65, :],
                                                 p_o[j][64:65, :])
                            nc.gpsimd.dma_start(rec_scr.ap()[j:j + 1, :],
                                                s_rec[64:65, :])
                            rec = misc.tile([64, 512], F32, tag=f'rec{j}',
                                            name=f'rec{j}')
                            nc.gpsimd.dma_start(
                                rec[:],
                                bass.AP(tensor=rec_scr.ap().tensor,
                                        offset=j * 512,
                                        ap=[[0, 64], [1, 512]]))
                            o_tmp = misc.tile([64, 512], F32, tag=f'otmp{j}',
                                              name=f'otmp{j}')
                            nc.vector.tensor_mul(o_tmp[:], p_o[j][0:64, :],
                                                 rec[:])
                            nc.gpsimd.dma_start(
                                o_sb[slab][po:po + 64, ssl], o_tmp[:])
                    # out projection + residual for this sequence half,
                    # overlapping the last head-pair's softmax tail (reuses
                    # the scores psum rings)
                    for ot in range(CT):
                        p = psc.tile([128, 512], F32,
                                     tag='sc0' if ot % 2 == 0 else 'sc1')
                        for ct in range(CT):
                            nc.tensor.matmul(
                                p[:], wo_sb[ct][:, ot * 128:(ot + 1) * 128],
                                o_sb[ct][:, ssl],
                                start=(ct == 0), stop=(ct == CT - 1))
                        nc.vector.scalar_tensor_tensor(
                            x2_sb[ot][:, ssl], p[:],
                            bo_t[:, ot:ot + 1],
                            x1_sb[ot][:, ssl],
                            op0=ALU.add, op1=ALU.add)

        # ================= Conv module =================
        # u (GLU output) is stored fp8 in ci-tile PAIRS for DoubleRow matmul:
        # u_pad[cp][:, i, :] = channel tile 2*cp+i, scaled by U_SCALE.
        with tc.tile_pool(name='upad', bufs=1) as up:
            u_pad = [up.tile([128, 2, S + 2 * PAD], F8, tag=f'u{i}', name=f'u{i}')
                     for i in range(UT // 2)]
            # zero the pads via tiny DRAM-sourced DMAs
            zp = din['zpad'].ap()
            for i in range(UT // 2):
                for sl in range(2):
                    for off in (0, S + PAD):
                        nc.sync.dma_start(
                            u_pad[i][:, sl, off:off + PAD],
                            bass.AP(tensor=zp.tensor, offset=0,
                                    ap=[[0, 128], [1, PAD]]))
            # pw1 + GLU
            with tc.tile_pool(name='wpw1', bufs=1) as wp1, \
                 tc.tile_pool(name='sig', bufs=2) as sigp, \
                 tc.tile_pool(name='ps_pw1', bufs=4, space='PSUM') as ps1:
                pw1_sb = []
                for i in range(CT):
                    t = wp1.tile([128, 2 * CONV_INNER], F32R, tag=f'pw1_{i}')
                    nc.sync.dma_start(t[:], din['w_pw1'].ap()[i * 128:(i + 1) * 128, :])
                    pw1_sb.append(t)
                bpw1_t = _bias_tile(nc, btp, din['b_pw1'], 2 * UT)
                for ut in range(UT):
                    for sc in range(SC):
                        p_a = ps1.tile([128, 512], F32, tag='pp')
                        p_g = ps1.tile([128, 512], F32, tag='pp')
                        for ct in range(CT):
                            nc.tensor.matmul(
                                p_a[:], pw1_sb[ct][:, ut * 128:(ut + 1) * 128],
                                x2_sb[ct][:, sc * 512:(sc + 1) * 512],
                                start=(ct == 0), stop=(ct == CT - 1))
                        for ct in range(CT):
                            nc.tensor.matmul(
                                p_g[:], pw1_sb[ct][:, CONV_INNER + ut * 128:CONV_INNER + (ut + 1) * 128],
                                x2_sb[ct][:, sc * 512:(sc + 1) * 512],
                                start=(ct == 0), stop=(ct == CT - 1))
                        sig = sigp.tile([128, 512], F32, tag='sig')
                        nc.scalar.activation(sig[:], p_g[:], AF.Sigmoid,
                                             bias=bpw1_t[:, UT + ut:UT + ut + 1])
                        nc.vector.scalar_tensor_tensor(
                            u_pad[ut // 2][:, ut % 2,
                                           PAD + sc * 512:PAD + (sc + 1) * 512],
                            p_a[:], bpw1_t[:, ut:ut + 1], sig[:],
                            op0=ALU.add, op1=ALU.mult)

            # dense conv1d over seq (K=31) + silu, fp8 DoubleRow: each matmul
            # contracts a 256-wide ci PAIR at double pump rate.
            h_sb = [hid_tile(i) for i in range(UT)]
            CP = UT // 2
            with tc.tile_pool(name='wdc', bufs=3) as wdc, \
                 tc.tile_pool(name='ps_dc', bufs=4, space='PSUM') as psd:
                bdc_t = _bias_tile(nc, btp, din['b_dc'], UT)
                for ot in range(UT):
                    ps_c = [psd.tile([128, 512], F32, tag='cv', name=f'cv{_sc}') for _sc in range(SC)]
                    for cp in range(CP):
                        wt = wdc.tile([128, K, 2, 128], F8, tag='dw')
                        nc.sync.dma_start(
                            wt[:], din['w_dc'].ap()[ot, cp].rearrange(
                                'p (k two c) -> p k two c', k=K, two=2))
                        for k in range(K):
                            for sc in range(SC):
                                nc.tensor.matmul(
                                    ps_c[sc][:], wt[:, k],
                                    u_pad[cp][:, :, k + sc * 512:k + sc * 512 + 512],
                                    start=(cp == 0 and k == 0),
                                    stop=(cp == CP - 1 and k == K - 1),
                                    perf_mode=PM_DR)
                    for sc in range(SC):
                        nc.scalar.activation(
                            h_sb[ot][:, sc * 512:(sc + 1) * 512], ps_c[sc][:],
                            AF.Silu, bias=bdc_t[:, ot:ot + 1],
                            scale=1.0 / (U_SCALE * W_SCALE))

        # pw2 + residual
        x3_sb = []
        with tc.tile_pool(name='wpw2', bufs=1) as wp2, \
             tc.tile_pool(name='ps_pw2', bufs=4, space='PSUM') as ps2:
            pw2_sb = []
            for i in range(UT):
                t = wp2.tile([128, DIM], F32R, tag=f'pw2_{i}')
                nc.sync.dma_start(t[:], din['w_pw2'].ap()[i * 128:(i + 1) * 128, :])
                pw2_sb.append(t)
            bpw2_t = _bias_tile(nc, btp, din['b_pw2'], CT)
            for ot in range(CT):
                t = new_resid(ot)
                for sc in range(SC):
                    p = ps2.tile([128, 512], F32, tag='pp')
                    for ct in range(UT):
                        nc.tensor.matmul(
                            p[:], pw2_sb[ct][:, ot * 128:(ot + 1) * 128],
                            h_sb[ct][:, sc * 512:(sc + 1) * 512],
                            start=(ct == 0), stop=(ct == UT - 1))
                    nc.vector.scalar_tensor_tensor(
                        t[:, sc * 512:(sc + 1) * 512], p[:],
                        bpw2_t[:, ot:ot + 1],
                        x2_sb[ot][:, sc * 512:(sc + 1) * 512],
                        op0=ALU.add, op1=ALU.add)
                x3_sb.append(t)

        # ================= FF2 (+ fused final affine & store) ============
        fing_t = _bias_tile(nc, btp, din['fin_g'], CT)
        finsc_t = _bias_tile(nc, btp, din['fin_sc'], CT)
        finbb_t = _bias_tile(nc, btp, din['fin_bb'], CT)
        ff_block(x3_sb, din['w_ff2_1'], din['b_ff2_1'],
                 din['w_ff2_2'], din['b_ff2_2'], 'ff2',
                 final=(fing_t, finsc_t, finbb_t, out_d))


_prog_cache = {}


def _get_program():
    if 'nc' not in _prog_cache:
        _prog_cache['nc'] = build_program()
    return _prog_cache['nc']


def kernel(**inputs):
    inputs = {k: np.asarray(v, dtype=np.float32) for k, v in inputs.items()}
    w = _host_prep(inputs)
    nc = _get_program()
    x = inputs['x'][..., 0]  # [B, DIM, S]
    in_maps = [dict(w, x=np.ascontiguousarray(x[b])) for b in range(N_CORES)]
    res = run_bass_kernel_spmd(nc, in_maps, core_ids=list(range(N_CORES)))
    out = np.stack([res.results[b]['out'] for b in range(N_CORES)])
    return out[..., None].astype(np.float32)

